# revision 1
# baseline (speedup 1.0000x reference)
"""DAHead (dual-attention head) Trainium2 kernel, v2.

8-core SPMD: core c handles sample c//2, spatial half c%2 (odd cores get
the sample vertically flipped so every core runs the same program; conv
weights are dy-flipped to match and the host un-flips the output half).

v2 strategy vs v1: the two cores of a sample pair now SPLIT the heavy
convs spatially instead of both computing the full image.  Each core
convolves only its own 34 rows (+1 halo row of x), computes q for its
own 34 rows and k/v for its own 32-row token half, then the pair
exchanges k/v via AllGather (softmax and the weighted sum are
permutation-invariant over the token axis j, so the gathered halves can
stay in each core's local frame - no un-flipping needed).  The CAM
global mean is assembled with a tiny pair AllReduce of per-channel
partial sums.

Numerics: fp32r matmuls (~13 effective mantissa bits, full PE rate)
replace the v1 bf16x2 3-pass scheme for both convs, the v-projection
and the 1x1 output projections; q/k projections stay fp32 and the big
[i x j] logits matmul runs as a packed fp16 hi/lo 2-pass (error ~2^-21,
needed because the softmax is argmax-like).  End-to-end rel err vs the
fp64 reference is ~5e-3 (gate 2e-2).
"""
import sys

if '/opt/trn_rl_repo' not in sys.path:
    sys.path.insert(0, '/opt/trn_rl_repo')

import numpy as np
import ml_dtypes

import concourse.bass as bass
import concourse.mybir as mybir
import concourse.tile as tile
from concourse import bacc
from concourse.bass_utils import run_bass_kernel_spmd

dt = mybir.dt
f32 = dt.float32
f32r = dt.float32r
bf16 = dt.bfloat16
fp16 = dt.float16
BF = ml_dtypes.bfloat16
AF = mybir.ActivationFunctionType
OP = mybir.AluOpType

C = 512          # channels
P = 128          # partition size
NCH = C // P     # channel chunks (4)
H = W = 64
HW = H * W       # 4096
CR = 64          # q/k channels
OC = 64          # output channels
OWN_ROWS = 34    # rows convolved per core (local frame rows 0..33)
OWN = OWN_ROWS * W    # 2176 = 17*128
XR = OWN_ROWS + 1     # x rows loaded (halo row 34 feeds conv row 33)
# x is stored zero-padded: top pad row + left/right pad cols, so every conv
# tap is a full even-width window (fp32r matmul ISA requires even moving
# dims) and SAME padding falls out of the zero border.
XPR = XR + 1          # 36 rows (row 0 = zeros)
XPW = W + 2           # 66 cols (cols 0 and 65 = zeros)
NIC = OWN // P        # 17 attention i-chunks
JROWS = 32            # token rows owned per core (disjoint pair cover)
JOWN = JROWS * W      # 2048
NJC_OWN = JOWN // P   # 16
NJC = HW // P         # 32 j-chunks after the gather
EPS = 1e-5
GROUPS = [[0, 1], [2, 3], [4, 5], [6, 7]]

# conv h-blocks over the 34 own rows; every block >= 4 rows so the fp32r
# matmul moving dim stays >= 256 (full PE rate)
HBS = [(0, 7), (7, 7), (14, 7), (21, 7), (28, 6)]

# tap order: full-coverage center tap first (needed for PSUM start flag)
_ALL = [(ci, dy, dx) for ci in range(NCH) for dy in (-1, 0, 1) for dx in (-1, 0, 1)]
TAPS = [(0, 0, 0)] + [t for t in _ALL if t != (0, 0, 0)]
NT = len(TAPS)   # 36

Q_EDGES = [0, 512, 1024, 1536, 2048, OWN]
K_EDGES = [0, 512, 1024, 1536, JOWN]


def _conv_tap_aps(psum_t, x_t, row0, rows, dy, dx):
    """APs for one conv tap on the h-block [row0, row0+rows).

    x_t is zero-bordered [128, XPR, XPW] (data at [1:, 1:65]), so every
    tap reads a full rows x 64 window - no edge clamps, even moving dims.
    """
    out_ap = psum_t[:, 0:rows, :]
    in_ap = x_t[:, row0 + 1 + dy: row0 + 1 + dy + rows, 1 + dx: 1 + dx + W]
    return out_ap, in_ap


def _emit_conv(nc, tc, x_t, w_d, scale_t, bias_t, store, pools):
    """3x3 conv over own rows in fp32r + BN + lrelu; store(co, f32_ap).

    Weight DMAs ride the Activation queue so they are not stuck behind
    data DMAs (k/v spills) on the sync queue.  The tile pools are shared
    between the PAM and CAM instances so the second conv's weight
    prefetch reuses the first conv's space (clean WAR, no dependency on
    anything downstream of the collectives).
    """
    HT = NT // 2  # weights stream in tap-halves (smaller tiles, deeper prefetch)
    with tc.tile_pool(name="wconv", bufs=2) as pw, \
         tc.tile_pool(name="conv_evac", bufs=2) as pe, \
         tc.tile_pool(name="ps_conv", bufs=1, space="PSUM") as psc:
        for co in range(NCH):
            if co == 0 and pools is not None:
                wh = [pools[:, 0:HT, :], pools[:, HT:NT, :]]
            else:
                wh = []
                for h in range(2):
                    wt = pw.tile([P, HT, P], f32r, tag="w", name="w")
                    nc.scalar.dma_start(out=wt,
                                        in_=w_d[co][:, h * HT:(h + 1) * HT, :])
                    wh.append(wt)
            pst = [psc.tile([P, rows, W], f32, tag=f"cv{b}", name=f"cv{b}")
                   for b, (row0, rows) in enumerate(HBS)]
            for t, (ci, dy, dx) in enumerate(TAPS):
                for b, (row0, rows) in enumerate(HBS):
                    o_ap, i_ap = _conv_tap_aps(pst[b], x_t[ci], row0, rows,
                                               dy, dx)
                    nc.tensor.matmul(o_ap, wh[t // HT][:, t % HT, :], i_ap,
                                     start=(t == 0), stop=(t == NT - 1))
            for b, (row0, rows) in enumerate(HBS):
                z = pe.tile([P, 7 * W], f32, tag="z", name="z")[:, 0:rows * W]
                nc.scalar.activation(
                    out=z, in_=pst[b].rearrange("p a b -> p (a b)"),
                    func=AF.Identity, bias=bias_t[co], scale=scale_t[co])
                store(co, row0 * W, rows * W, z)


def _emit_qkv(nc, tc, d, ct, f_t, kin, vin):
    """q (own rows, fp32), k/v (own token half) -> DRAM, pair AllGather.

    k is hi/lo-split to fp16 BEFORE the gather, so the post-gather work
    is pure DMA (no DVE) and nothing downstream of the collective sits on
    an SBUF range the CAM conv wants to reuse.  q is packed right here
    too (local, no gather involved).
    """
    with tc.tile_pool(name="qk_w", bufs=1) as pqw, \
         tc.tile_pool(name="v_ev", bufs=2) as pve, \
         tc.tile_pool(name="ps_qkv", bufs=2, space="PSUM") as psq:
        wq_t = [pqw.tile([P, CR], f32, name=f"wq{i}", tag=f"wq{i}") for i in range(NCH)]
        wk_t = [pqw.tile([P, CR], f32, name=f"wk{i}", tag=f"wk{i}") for i in range(NCH)]
        wv_t = [pqw.tile([P, C], f32r, name=f"wv{i}", tag=f"wv{i}") for i in range(NCH)]
        bv_t = pqw.tile([P, C], f32, name="bv_t", tag="bv_t")
        nc.sync.dma_start(out=bv_t, in_=d['bv'].to_broadcast([P, C]))
        for i in range(NCH):
            nc.sync.dma_start(out=wq_t[i], in_=d['wq'][i])
            nc.sync.dma_start(out=wk_t[i], in_=d['wk'][i])
            nc.sync.dma_start(out=wv_t[i], in_=d['wv'][i])

        def proj(dst, wts, bias_t, edges):
            for bi in range(len(edges) - 1):
                off, end = edges[bi], edges[bi + 1]
                sz = end - off
                pq = psq.tile([CR, 512], f32, tag="pq", name="pq")[:, 0:sz]
                for ci in range(NCH):
                    nc.tensor.matmul(pq, wts[ci], f_t[ci][:, off:end].bitcast(f32),
                                     start=(ci == 0), stop=(ci == NCH - 1))
                nc.scalar.activation(out=dst[:, off:end], in_=pq,
                                     func=AF.Identity, bias=bias_t, scale=1.0)

        q32 = ct['q32']
        k32 = ct['k32']
        qpk, khd, klo = ct['qpk'], ct['khd'], ct['klo']

        proj(k32, wk_t, ct['bk'], K_EDGES)
        # stage the local hi/lo split in the (long-lived) khd/klo tiles;
        # the post-gather unpack rewrites both halves anyway
        nc.vector.tensor_copy(out=khd[0:CR, 0:JOWN], in_=k32)
        nc.vector.tensor_sub(klo[:, 0:JOWN], k32, khd[0:CR, 0:JOWN])
        nc.sync.dma_start(out=kin[:, 0, :], in_=khd[0:CR, 0:JOWN])
        nc.sync.dma_start(out=kin[:, 1, :], in_=klo[:, 0:JOWN])
        nc.gpsimd.collective_compute(
            "AllGather", mybir.AluOpType.bypass, replica_groups=GROUPS,
            ins=[kin.opt()], outs=[ct['kout'].opt()])

        for jc in range(NJC_OWN):
            pv = psq.tile([P, C], f32, tag="pv", name="pv")
            s = jc * P
            for ci in range(NCH):
                nc.tensor.matmul(pv, f_t[ci][:, s:s + P], wv_t[ci],
                                 start=(ci == 0), stop=(ci == NCH - 1))
            vtmp = pve.tile([P, C], fp16, tag="vtmp", name="vtmp")
            nc.vector.tensor_add(vtmp, pv, bv_t)
            nc.sync.dma_start(out=vin[:, jc, :], in_=vtmp)
        nc.gpsimd.collective_compute(
            "AllGather", mybir.AluOpType.bypass, replica_groups=GROUPS,
            ins=[vin.opt()], outs=[ct['vout'].opt()])

        proj(q32, wq_t, ct['bq'], Q_EDGES)
        qlo = ct['qlo']
        nc.vector.tensor_copy(out=qpk[0:CR, :], in_=q32)
        nc.vector.tensor_sub(qlo, q32, qpk[0:CR, :])


def _emit_qk_unpack(nc, ct):
    """Post-gather k unpack + q lo move: pure DMAs, emitted after the CAM
    conv and on the ACT queue, so nothing that waits on the k gather sits
    ahead of the v spills or conv weight loads."""
    nc.scalar.dma_start(out=ct['khd'][0:CR, 0:JOWN], in_=ct['kout'][0, :, 0])
    nc.scalar.dma_start(out=ct['khd'][0:CR, JOWN:HW], in_=ct['kout'][1, :, 0])
    nc.scalar.dma_start(out=ct['klo'][:, 0:JOWN], in_=ct['kout'][0, :, 1])
    nc.scalar.dma_start(out=ct['klo'][:, JOWN:HW], in_=ct['kout'][1, :, 1])
    nc.scalar.dma_start(out=ct['khd'][CR:P, :], in_=ct['khd'][0:CR, :])
    nc.scalar.dma_start(out=ct['qpk'][CR:P, :], in_=ct['qlo'])


def _emit_attention(nc, tc, ct, f16_t, pam_sb, vt_t, ibs, post_cb=None):
    with tc.tile_pool(name="ls", bufs=2) as pls, \
         tc.tile_pool(name="ls16", bufs=2) as pls16, \
         tc.tile_pool(name="et", bufs=1) as pet, \
         tc.tile_pool(name="att_tmp", bufs=2) as pat, \
         tc.tile_pool(name="res_t", bufs=2) as prs, \
         tc.tile_pool(name="ps_l", bufs=2, space="PSUM") as psl, \
         tc.tile_pool(name="ps_t", bufs=2, space="PSUM") as pstp, \
         tc.tile_pool(name="ps_a", bufs=2, space="PSUM") as psa, \
         tc.tile_pool(name="ps_p", bufs=1, space="PSUM") as psp:
        qpk, khd, klo = ct['qpk'], ct['khd'], ct['klo']

        def softmax_tail(ls, e16, et_t, ph, rr2):
            """row max -> unnormalized exp pieces -> transposes; emitted one
            chunk late so the whole chain overlaps the next chunk's logit
            stream and never blocks its PSUM evacs on the DVE queue."""
            nmax8 = pat.tile([P, 8], f32, tag="nmax8", name="nmax8")
            for jb in range(HW // 512):
                nc.vector.tensor_reduce(
                    out=nmax8[:, jb:jb + 1], in_=ls[:, jb * 512:(jb + 1) * 512],
                    axis=mybir.AxisListType.X, op=OP.max)
            nmax = pat.tile([P, 1], f32, tag="nmax", name="nmax")
            nc.vector.tensor_reduce(out=nmax, in_=nmax8,
                                    axis=mybir.AxisListType.X,
                                    op=OP.max, negate=True)
            rsum8 = pat.tile([P, 8], f32, tag="rsum8", name="rsum8")
            for jb in range(HW // 512):
                nc.scalar.activation(
                    out=e16[:, jb * 512:(jb + 1) * 512],
                    in_=ls[:, jb * 512:(jb + 1) * 512], func=AF.Exp,
                    bias=nmax, scale=1.0, accum_out=rsum8[:, jb:jb + 1])
            for jb in range(HW // 512):
                pt = pstp.tile([P, 4 * P], fp16, tag="pt", name="pt")
                for k in range(4):
                    jc = 4 * jb + k
                    nc.tensor.transpose(
                        pt[:, k * P:(k + 1) * P],
                        e16[:, jc * P:(jc + 1) * P], ct['ident'])
                nc.vector.tensor_copy(
                    out=et_t[:, 4 * jb:4 * jb + 4, ph * P:(ph + 1) * P],
                    in_=pt.rearrange("p (a b) -> p a b", b=P))
            rsum = pat.tile([P, 1], f32, tag="rsum", name="rsum")
            nc.vector.tensor_reduce(out=rsum, in_=rsum8,
                                    axis=mybir.AxisListType.X, op=OP.add)
            with nc.allow_low_precision(
                    reason="1/rsum at fp16 only rescales whole rows"):
                nc.vector.reciprocal(out=rr2[:, ph:ph + 1], in_=rsum)

        for ib in ibs:
            ics = [2 * ib, 2 * ib + 1]
            if ics[-1] >= NIC:
                ics = ics[:1]
            isz = P * len(ics)
            ioff = ics[0] * P
            et_t = pet.tile([P, NJC, 2 * P], fp16, tag="et", name="et")
            rr2 = pat.tile([P, 2], fp16, tag="rr2", name="rr2")
            pending = None
            for ph, ic in enumerate(ics):
                ls = pls.tile([P, HW], f32, tag="ls", name="ls")
                for jb in range(HW // 512):
                    pl = psl.tile([P, 512], f32, tag="pl", name="pl")
                    nc.tensor.matmul(
                        pl, qpk[:, ic * P:(ic + 1) * P],
                        khd[:, jb * 512:(jb + 1) * 512], start=True, stop=False)
                    nc.tensor.matmul(
                        pl, qpk[0:CR, ic * P:(ic + 1) * P],
                        klo[:, jb * 512:(jb + 1) * 512], start=False, stop=True)
                    nc.scalar.activation(
                        out=ls[:, jb * 512:(jb + 1) * 512], in_=pl,
                        func=AF.Identity, bias=0.0, scale=1.0)
                if pending is not None:
                    pending()
                e16 = pls16.tile([P, HW], fp16, tag="e16", name="e16")
                pending = (lambda ls=ls, e16=e16, ph=ph:
                           softmax_tail(ls, e16, et_t, ph, rr2))
            pending()
            # broadcast 1/rsum (per i) to every partition via a transposed
            # row and a DRAM broadcast read; lands well before it is needed
            nics = len(ics)
            prt = psp.tile([2, P], fp16, tag="rrT", name="rrT")
            nc.tensor.transpose(prt[0:nics, :], rr2[:, 0:nics], ct['ident'])
            rrow = pat.tile([2, P], fp16, tag="rrow", name="rrow")
            nc.vector.tensor_copy(out=rrow[0:nics, :], in_=prt[0:nics, :])
            rrd = ct['rrd']
            nc.sync.dma_start(out=rrd[0, 0:nics, :], in_=rrow[0:nics, :])
            rb = pat.tile([P, 2 * P], fp16, tag="rb", name="rb")[:, 0:isz]
            nc.sync.dma_start(
                out=rb, in_=rrd.rearrange("a b c -> a (b c)")[:, 0:isz]
                .to_broadcast([P, isz]))
            pp = psp.tile([OC, 2 * P], f32, tag="pp", name="pp")[:, 0:isz]
            for co in range(NCH):
                pa = psa.tile([P, 2 * P], f32, tag="pa", name="pa")[:, 0:isz]
                for jc in range(NJC):
                    nc.tensor.matmul(
                        pa, vt_t[:, jc, co * P:(co + 1) * P],
                        et_t[:, jc, 0:isz],
                        start=(jc == 0), stop=(jc == NJC - 1))
                rt = prs.tile([P, 2 * P], f32r, tag="rt", name="rt")[:, 0:isz]
                nc.vector.tensor_mul(rt, pa, rb)
                # alpha is folded into wpoa on the host; the +f residual is
                # a second fp16 matmul accumulated into the same PSUM
                nc.tensor.matmul(pp, ct['wpoa'][co], rt,
                                 start=(co == 0), stop=False)
                nc.tensor.matmul(pp, ct['wpo16'][co],
                                 f16_t[co][:, ioff:ioff + isz],
                                 start=False, stop=(co == NCH - 1))
            nc.scalar.activation(out=pam_sb[:, ioff:ioff + isz], in_=pp,
                                 func=AF.Identity, bias=ct['bpo'], scale=1.0)
            if post_cb and ib in post_cb:
                post_cb[ib]()


def _emit_cam_mlp(nc, tc, d, ct, g_t, zc_full):
    """channel-attention MLP (on the pair-reduced mean) + 1x1 out-proj.

    Emitted between the two attention halves so its matmuls overlap the
    attention stream and its (tiny) wait on the mean AllReduce is hidden.
    Writes the CAM branch output into zc_full; the add into pam_sb
    happens after the second attention half.
    """
    with tc.tile_pool(name="mlp", bufs=1) as pm, \
         tc.tile_pool(name="ps_mlp", bufs=2, space="PSUM") as psm, \
         tc.tile_pool(name="ps_co", bufs=2, space="PSUM") as psco:
        msum = [pm.tile([P, 1], f32, name=f"ms{i}", tag=f"ms{i}") for i in range(NCH)]
        for i in range(NCH):
            nc.sync.dma_start(out=msum[i], in_=ct['mout'][i])
        wc1_t = [pm.tile([P, CR], f32, name=f"w1{i}", tag=f"w1{i}") for i in range(NCH)]
        wc2_t = [pm.tile([CR, P], f32, name=f"w2{i}", tag=f"w2{i}") for i in range(NCH)]
        wco_t = [pm.tile([P, OC], f32, name=f"wo{i}", tag=f"wo{i}") for i in range(NCH)]
        bc2_t = [pm.tile([P, 1], f32, name=f"b2{i}", tag=f"b2{i}") for i in range(NCH)]
        for i in range(NCH):
            nc.sync.dma_start(out=wc1_t[i], in_=d['wc1'][i])
            nc.sync.dma_start(out=wc2_t[i], in_=d['wc2'][i])
            nc.sync.dma_start(out=wco_t[i], in_=d['wco'][i])
            nc.sync.dma_start(out=bc2_t[i], in_=d['bc2'][i])
        p1 = psm.tile([CR, 1], f32, tag="p1", name="p1")
        for ci in range(NCH):
            nc.tensor.matmul(p1, wc1_t[ci], msum[ci],
                             start=(ci == 0), stop=(ci == NCH - 1))
        t1 = pm.tile([CR, 1], f32, name="t1", tag="t1")
        nc.scalar.activation(out=t1, in_=p1, func=AF.Identity,
                             bias=ct['bc1'], scale=1.0)
        y1 = pm.tile([CR, 1], f32, name="y1", tag="y1")
        nc.vector.scalar_tensor_tensor(out=y1, in0=t1, scalar=0.2, in1=t1,
                                       op0=OP.mult, op1=OP.max)
        wce = [pm.tile([P, OC], fp16, name=f"we{i}", tag=f"we{i}") for i in range(NCH)]
        for co in range(NCH):
            p2 = psm.tile([P, 1], f32, tag="p2", name="p2")
            nc.tensor.matmul(p2, wc2_t[co], y1, start=True, stop=True)
            s_t = pm.tile([P, 1], f32, name=f"s{co}", tag=f"s{co}")
            nc.scalar.activation(out=s_t, in_=p2, func=AF.Sigmoid,
                                 bias=bc2_t[co], scale=1.0)
            nc.vector.tensor_scalar_mul(wce[co], wco_t[co], s_t)
        for bi in range(len(Q_EDGES) - 1):
            off, end = Q_EDGES[bi], Q_EDGES[bi + 1]
            sz = end - off
            pco = psco.tile([OC, 512], f32, tag="pco", name="pco")[:, 0:sz]
            for ci in range(NCH):
                nc.tensor.matmul(pco, wce[ci], g_t[ci][:, off:end],
                                 start=(ci == 0), stop=(ci == NCH - 1))
            nc.scalar.activation(out=zc_full[:, off:end], in_=pco,
                                 func=AF.Identity, bias=ct['bco'], scale=1.0)


UPR = 4   # output row-pairs per upsample chunk
UPN = UPR + 2  # max su rows a chunk reads


def _emit_up_chunk(nc, pu, pam_sb, zc_full, y_d, r0, r1, addlo, addhi):
    """CAM add for pam rows [addlo,addhi) + bilinear x2 of su rows giving
    output rows [2*r0, 2*r1).  Needs su rows [r0-1, r1] complete (i.e. the
    attention writes AND the cam adds for those rows)."""
    ad = pam_sb[:, addlo * W:addhi * W]
    nc.vector.tensor_add(ad, ad, zc_full[:, addlo * W:addhi * W])
    su = pam_sb.rearrange("p (a b) -> p a b", b=W)  # [OC,34,64]
    in_lo = max(r0 - 1, 0)
    n = r1 - in_lo + 1          # su rows [in_lo, r1]
    m = r1 - r0
    base = r0 - in_lo
    a_t = pu.tile([OC, UPN, W], f32, name="a_t", tag="a_t")
    b_t = pu.tile([OC, UPN, W], f32, name="b_t", tag="b_t")
    seg = pam_sb[:, in_lo * W:(r1 + 1) * W]
    nc.vector.tensor_scalar_mul(
        a_t.rearrange("p a b -> p (a b)")[:, 0:n * W], seg, 0.75)
    nc.vector.tensor_scalar_mul(
        b_t.rearrange("p a b -> p (a b)")[:, 0:n * W], seg, 0.25)
    sh = pu.tile([OC, UPN, W, 2], f32, name="sh", tag="sh")
    nc.vector.tensor_copy(out=sh[:, 0:n, 0, 0], in_=su[:, in_lo:r1 + 1, 0])
    nc.vector.tensor_add(sh[:, 0:n, 1:W, 0], b_t[:, 0:n, 0:W - 1],
                         a_t[:, 0:n, 1:W])
    nc.vector.tensor_add(sh[:, 0:n, 0:W - 1, 1], a_t[:, 0:n, 0:W - 1],
                         b_t[:, 0:n, 1:W])
    nc.vector.tensor_copy(out=sh[:, 0:n, W - 1, 1], in_=su[:, in_lo:r1 + 1, W - 1])
    au = pu.tile([OC, UPN, 2 * W], f32, name="au", tag="au")
    bu = pu.tile([OC, UPN, 2 * W], f32, name="bu", tag="bu")
    shf = sh.rearrange("p a b c -> p a (b c)")
    nc.vector.tensor_scalar_mul(
        au.rearrange("p a b -> p (a b)")[:, 0:n * 2 * W],
        shf.rearrange("p a b -> p (a b)")[:, 0:n * 2 * W], 0.75)
    nc.vector.tensor_scalar_mul(
        bu.rearrange("p a b -> p (a b)")[:, 0:n * 2 * W],
        shf.rearrange("p a b -> p (a b)")[:, 0:n * 2 * W], 0.25)
    out_t = pu.tile([OC, UPR, 2, 2 * W], f32, name="out_t", tag="out_t")
    j0 = 1 if r0 == 0 else 0
    if r0 == 0:
        nc.vector.tensor_copy(out=out_t[:, 0, 0, :], in_=shf[:, 0, :])
    nc.vector.tensor_add(out_t[:, j0:m, 0, :],
                         bu[:, base + j0 - 1:base + m - 1, :],
                         au[:, base + j0:base + m, :])
    nc.vector.tensor_add(out_t[:, 0:m, 1, :], au[:, base:base + m, :],
                         bu[:, base + 1:base + m + 1, :])
    nc.sync.dma_start(
        out=y_d[:, 2 * r0:2 * r1, :],
        in_=out_t[:, 0:m, :, :].rearrange("p a b c -> p (a b) c"))


def _build():
    nc = bacc.Bacc("TRN2", target_bir_lowering=False, debug=False,
                   enable_asserts=True, num_devices=8)

    def din(name, shape, dtp=f32):
        return nc.dram_tensor(name, shape, dtp, kind="ExternalInput").ap()

    d = {
        'x': din("x", [NCH, P, XPR, XPW], f32r),
        'wp': din("wp", [NCH, P, NT, P], f32r),
        'wc': din("wc", [NCH, P, NT, P], f32r),
        'sp': din("sp", [NCH, P, 1]), 'bp': din("bp", [NCH, P, 1]),
        'sc': din("sc", [NCH, P, 1]), 'bc': din("bc", [NCH, P, 1]),
        'wq': din("wq", [NCH, P, CR]), 'wk': din("wk", [NCH, P, CR]),
        'bq': din("bq", [CR, 1]), 'bk': din("bk", [CR, 1]),
        'wv': din("wv", [NCH, P, C], f32r),
        'bv': din("bv", [1, C]),
        'wpoa': din("wpoa", [NCH, P, OC], f32r),
        'wpo16': din("wpo16", [NCH, P, OC], fp16),
        'bpo': din("bpo", [OC, 1]),
        'wc1': din("wc1", [NCH, P, CR]), 'bc1': din("bc1", [CR, 1]),
        'wc2': din("wc2", [NCH, CR, P]), 'bc2': din("bc2", [NCH, P, 1]),
        'wco': din("wco", [NCH, P, OC]), 'bco': din("bco", [OC, 1]),
        'ident': din("ident", [P, P], fp16),
    }
    y_d = nc.dram_tensor("y", [OC, H, 2 * W], f32, kind="ExternalOutput").ap()

    with tile.TileContext(nc) as tc:
        with tc.tile_pool(name="consts", bufs=2) as pc, \
             tc.tile_pool(name="dram", bufs=1, space="DRAM") as pfd, \
             tc.tile_pool(name="pam_out", bufs=1) as p_pam, \
             tc.tile_pool(name="qk_sb", bufs=1) as pqs:
            pam_sb = p_pam.tile([OC, OWN], f32, name="pam_sb", tag="pam_sb")
            zc_full = p_pam.tile([OC, OWN], f32, name="zc_full", tag="zc_full")
            # consts ride the DVE queue so the x loads own the sync queue
            # from t=0 (PE start gates on x chunk 0 + first conv weights)
            ct = {}
            ct['ident'] = pc.tile([P, P], fp16, name="ident", tag="ident")
            nc.gpsimd.dma_start(out=ct['ident'], in_=d['ident'])
            for nm, rows in (('bq', CR), ('bk', CR), ('bpo', OC), ('bco', OC),
                             ('bc1', CR)):
                ct[nm] = pc.tile([rows, 1], f32, name=f"{nm}_t", tag=f"{nm}_t")
                nc.gpsimd.dma_start(out=ct[nm], in_=d[nm])
            for nm in ('sp', 'bp', 'sc', 'bc'):
                ct[nm] = [pc.tile([P, 1], f32, name=f"{nm}{i}_t", tag=f"{nm}{i}_t")
                          for i in range(NCH)]
                for i in range(NCH):
                    nc.gpsimd.dma_start(out=ct[nm][i], in_=d[nm][i])
            ct['wpoa'] = [pc.tile([P, OC], f32r, name=f"wpa{i}_t", tag=f"wpa{i}_t")
                          for i in range(NCH)]
            ct['wpo16'] = [pc.tile([P, OC], fp16, name=f"wp6{i}_t",
                           tag=f"wp6{i}_t") for i in range(NCH)]
            for i in range(NCH):
                nc.gpsimd.dma_start(out=ct['wpoa'][i], in_=d['wpoa'][i])
                nc.gpsimd.dma_start(out=ct['wpo16'][i], in_=d['wpo16'][i])

            kin = pfd.tile([CR, 2, JOWN], fp16, name="kin", tag="kin")
            ct['kout'] = pfd.tile([2, CR, 2, JOWN], fp16, name="kout", tag="kout")
            vin = pfd.tile([P, NJC_OWN, C], fp16, name="vin", tag="vin")
            ct['vout'] = pfd.tile([2, P, NJC_OWN, C], fp16, name="vout", tag="vout")
            min_d = pfd.tile([NCH, P, 1], f32, name="min_d", tag="min_d")
            ct['mout'] = pfd.tile([NCH, P, 1], f32, name="mout", tag="mout")
            ct['rrd'] = pfd.tile([1, 2, P], fp16, name="rrd", tag="rrd")

            # SBUF-resident q/k packs (live into attention)
            ct['qpk'] = pqs.tile([P, OWN], fp16, name="qpk", tag="qpk")
            ct['khd'] = pqs.tile([P, HW], fp16, name="khd", tag="khd")
            ct['klo'] = pqs.tile([CR, HW], fp16, name="klo", tag="klo")

            with tc.tile_pool(name="g_store", bufs=1) as p_g, \
                 tc.tile_pool(name="f16_store", bufs=1) as p_f16:
                g_t = [p_g.tile([P, OWN], fp16, name=f"g{i}", tag=f"g{i}")
                       for i in range(NCH)]
                f16_t = [p_f16.tile([P, OWN], fp16, name=f"h{i}", tag=f"h{i}")
                         for i in range(NCH)]

                with tc.tile_pool(name="q32_pool", bufs=1) as pq32:
                    ct['q32'] = pq32.tile([CR, OWN], f32, name="q32", tag="q32")
                    ct['k32'] = pq32.tile([CR, JOWN], f32, name="k32", tag="k32")
                    ct['qlo'] = pq32.tile([CR, OWN], fp16, name="qlo", tag="qlo")

                    with tc.tile_pool(name="f_store", bufs=1) as p_f, \
                         tc.tile_pool(name="xs", bufs=1) as px:
                        f_t = [p_f.tile([P, OWN], f32r, name=f"f{i}", tag=f"f{i}")
                               for i in range(NCH)]
                        x_t = [px.tile([P, XPR, XPW], f32r, name=f"x{i}",
                                       tag=f"x{i}")
                               for i in range(NCH)]
                        for i in range(NCH):
                            nc.sync.dma_start(out=x_t[i], in_=d['x'][i])

                        def f_store(co, off, ln, z):
                            nc.vector.scalar_tensor_tensor(
                                out=f_t[co][:, off:off + ln], in0=z, scalar=0.2,
                                in1=z, op0=OP.mult, op1=OP.max)

                        def g_store(co, off, ln, z):
                            nc.vector.scalar_tensor_tensor(
                                out=g_t[co][:, off:off + ln], in0=z, scalar=0.2,
                                in1=z, op0=OP.mult, op1=OP.max)

                        _emit_conv(nc, tc, x_t, d['wp'], ct['sp'], ct['bp'],
                                   f_store, None)
                        # prefetch the CAM conv's first weight tile now, in
                        # space (x pool) with no dependency on the gathers
                        cam_w0 = px.tile([P, NT, P], f32r, name="cw0",
                                         tag="cw0")
                        nc.scalar.dma_start(out=cam_w0, in_=d['wc'][0])
                        _emit_qkv(nc, tc, d, ct, f_t, kin, vin)
                        # residual copy of f in fp16 (frees the fp32 f
                        # before attention; q/k/v already consumed fp32)
                        for i in range(NCH):
                            nc.vector.tensor_copy(out=f16_t[i],
                                                  in_=f_t[i].bitcast(f32))
                        _emit_conv(nc, tc, x_t, d['wc'], ct['sc'], ct['bc'],
                                   g_store, cam_w0)
                        _emit_qk_unpack(nc, ct)

                    # CAM partial mean over own token half + pair AllReduce
                    with tc.tile_pool(name="msum_p", bufs=1) as pms:
                        for i in range(NCH):
                            ms = pms.tile([P, 1], f32, name=f"msp{i}",
                                          tag=f"msp{i}")
                            nc.vector.tensor_reduce(
                                out=ms, in_=g_t[i][:, 0:JOWN],
                                axis=mybir.AxisListType.X, op=OP.add)
                            nc.sync.dma_start(out=min_d[i], in_=ms)
                        nc.gpsimd.collective_compute(
                            "AllReduce", mybir.AluOpType.add,
                            replica_groups=GROUPS,
                            ins=[min_d.opt()], outs=[ct['mout'].opt()])

                with tc.tile_pool(name="vt2", bufs=1) as pv2, \
                     tc.tile_pool(name="up", bufs=1) as pu:
                    vt_t = pv2.tile([P, NJC, C], fp16, name="vt2_t", tag="vt2_t")
                    nc.sync.dma_start(out=vt_t[:, 0:NJC_OWN, :], in_=ct['vout'][0])
                    nc.sync.dma_start(out=vt_t[:, NJC_OWN:NJC, :],
                                      in_=ct['vout'][1])
                    n_blocks = (NIC + 1) // 2

                    def upc(k):
                        r0, r1 = 4 * k, 4 * k + 4
                        alo = 0 if k == 0 else 4 * k + 1
                        ahi = OWN_ROWS if k == 7 else 4 * k + 5
                        _emit_up_chunk(nc, pu, pam_sb, zc_full, y_d,
                                       r0, r1, alo, ahi)

                    def ups(*ks):
                        return lambda: [upc(k) for k in ks]

                    _emit_attention(nc, tc, ct, f16_t, pam_sb, vt_t,
                                    list(range(0, 5)))
                    _emit_cam_mlp(nc, tc, d, ct, g_t, zc_full)
                    _emit_attention(nc, tc, ct, f16_t, pam_sb, vt_t,
                                    list(range(5, n_blocks)),
                                    post_cb={5: ups(0, 1), 6: ups(2, 3),
                                             7: ups(4, 5, 6)})
                    upc(7)
    nc.compile()
    return nc


_NC_CACHE = None


def _get_nc():
    global _NC_CACHE
    if _NC_CACHE is None:
        _NC_CACHE = _build()
    return _NC_CACHE


_TAP_CI = np.array([t[0] for t in TAPS])
_TAP_DY = np.array([t[1] + 1 for t in TAPS])
_TAP_DX = np.array([t[2] + 1 for t in TAPS])


def _pack_conv(wfull):
    """[C, C, 3, 3] -> [NCH(co), P(ci_local), NT, P(co_local)] lhsT tiles."""
    wr = np.asarray(wfull, np.float32).reshape(NCH, P, NCH, P, 3, 3)
    wt = wr.transpose(0, 2, 4, 5, 3, 1)  # [co, ci, dy, dx, ci_l, co_l]
    taps = wt[:, _TAP_CI, _TAP_DY, _TAP_DX]  # [co, NT, ci_l, co_l]
    return np.ascontiguousarray(taps.transpose(0, 2, 1, 3))


def _packT(w, free):
    """w [free, C] -> [NCH, P, free] lhsT chunks."""
    return np.ascontiguousarray(np.asarray(w, np.float32).T.reshape(NCH, P, free))


def _prep_shared(inputs, flip):
    wp = np.asarray(inputs['W_pam_in'], np.float32)
    wc = np.asarray(inputs['W_cam_in'], np.float32)
    if flip:
        wp = wp[:, :, ::-1, :]
        wc = wc[:, :, ::-1, :]

    def bnfold(g, b, m, v):
        s = (np.asarray(g, np.float32)
             / np.sqrt(np.asarray(v, np.float32) + EPS)).astype(np.float32)
        bb = (np.asarray(b, np.float32)
              - np.asarray(m, np.float32) * s).astype(np.float32)
        return s.reshape(NCH, P, 1), bb.reshape(NCH, P, 1)

    sp, bp = bnfold(inputs['pam_gamma'], inputs['pam_beta'],
                    inputs['pam_mean'], inputs['pam_var'])
    sc, bc = bnfold(inputs['cam_gamma'], inputs['cam_beta'],
                    inputs['cam_mean'], inputs['cam_var'])
    # Wc2 [C, CR] -> lhsT chunks [NCH, CR, P]
    wc2 = np.ascontiguousarray(
        np.asarray(inputs['Wc2'], np.float32).reshape(NCH, P, CR).transpose(0, 2, 1))
    return {
        'wp': _pack_conv(wp), 'wc': _pack_conv(wc),
        'sp': sp, 'bp': bp, 'sc': sc, 'bc': bc,
        'wq': _packT(inputs['Wq'], CR), 'wk': _packT(inputs['Wk'], CR),
        'bq': np.asarray(inputs['bq'], np.float32).reshape(CR, 1),
        'bk': np.asarray(inputs['bk'], np.float32).reshape(CR, 1),
        'wv': _packT(inputs['Wv'], C),
        'bv': np.asarray(inputs['bv'], np.float32).reshape(1, C),
        'wpoa': np.float32(inputs['alpha'][0]) * _packT(inputs['W_pam_out'], OC),
        'wpo16': _packT(inputs['W_pam_out'], OC).astype(np.float16),
        'bpo': np.asarray(inputs['b_pam_out'], np.float32).reshape(OC, 1),
        'wc1': _packT(np.asarray(inputs['Wc1'], np.float32) / HW, CR),
        'bc1': np.asarray(inputs['bc1'], np.float32).reshape(CR, 1),
        'wc2': wc2,
        'bc2': np.asarray(inputs['bc2'], np.float32).reshape(NCH, P, 1),
        'wco': _packT(inputs['W_cam_out'], OC),
        'ident': np.eye(P, dtype=np.float16),
        'bco': np.asarray(inputs['b_cam_out'], np.float32).reshape(OC, 1),
    }


def _make_in_maps(inputs):
    x = np.asarray(inputs['x'], np.float32)  # [4, 512, 64, 64]
    shared = {f: _prep_shared(inputs, f) for f in (False, True)}
    in_maps = []
    for c in range(8):
        s, flip = c // 2, c % 2
        xs = x[s]
        if flip:
            xs = xs[:, ::-1, :]
        xp = np.zeros((C, XPR, XPW), np.float32)
        xp[:, 1:XPR, 1:W + 1] = xs[:, 0:XR, :]
        m = dict(shared[bool(flip)])
        m['x'] = np.ascontiguousarray(xp.reshape(NCH, P, XPR, XPW))
        in_maps.append(m)
    return in_maps


def kernel(**inputs):
    nc = _get_nc()
    in_maps = _make_in_maps(inputs)
    res = run_bass_kernel_spmd(nc, in_maps, list(range(8)))
    out = np.empty((4, OC, 2 * H, 2 * W), np.float32)
    for c in range(8):
        s, flip = c // 2, c % 2
        o = res.results[c]['y']  # [64, 64, 128]
        if flip:
            out[s, :, H:2 * H, :] = o[:, ::-1, :]
        else:
            out[s, :, 0:H, :] = o
    return out

